# revision 1
# baseline (speedup 1.0000x reference)
"""Trainium2 Bass kernel for nn_MultiHeadedAttention_33835752358170.

Shapes (hardcoded): x [4, 2048, 1024] f32, w_in [192, 1024], b_in [192],
w_out [1024, 64], b_out [1024].  Module quirk: d_k = 64 total across 16
heads -> head_dim = 4.  Scale is 1/sqrt(64) = 1/8, folded into the q
projection weights on the host.

Sharding: 8 cores = 4 batches x 2 query-halves.  Each core computes
K/V over its batch's full sequence (S=2048) and attention + output
projection for its own 1024 query rows.

Per-core kernel layout choices:
- scores computed TRANSPOSED: S^T[l, sq] tiles [128, 1024] in PSUM via
  K=4 matmuls (per-head).  q^T/k^T live in "strip" layout: head h=4j+s
  occupies partitions [32s, 32s+4) of strip-tile j (satisfies the
  tile_position base-partition rule and enables PE row-group overlap).
- exp on ScalarE reads PSUM scores, writes bf16 SBUF (the only
  transcendental engine; this is the bottleneck ~250us).
- A@V via col-tiled matmuls: head h=4j+s has a private 32-wide lhsT
  window in v_aug (v dims at cols 8s..8s+3, ones col at 8s+4, rest 0);
  out accumulates at PSUM partitions 32j+8s+d over all 16 l-chunks.
  The ones column yields softmax denominators for free.
- normalization via 0/1 select/replicate matmuls + DVE reciprocal,
  then final projection with a slot-permuted w_out (host-built).
"""

import math

import numpy as np
import ml_dtypes

import concourse.bass as bass
import concourse.mybir as mybir
import concourse.tile as tile
from concourse import bacc
from concourse.bass_utils import run_bass_kernel_spmd

BF16 = ml_dtypes.bfloat16
F32 = np.float32

B, S, DM = 4, 2048, 1024
NH, DK = 16, 64
HD = 4          # head dim
SQ = 1024       # query rows per core
NC_CORES = 8

_cache = {}


def _slot(h):
    # head h = 4j+s -> output partition base 32j + 8s (+d, denom at +4)
    j, s = divmod(h, 4)
    return 32 * j + 8 * s


def _build_nc():
    f32 = mybir.dt.float32
    bf16 = mybir.dt.bfloat16
    Exp = mybir.ActivationFunctionType.Exp

    nc = bacc.Bacc("TRN2", target_bir_lowering=False, debug=False)

    # ---- DRAM I/O ----
    d_xT = nc.dram_tensor("xT", [DM, S], bf16, kind="ExternalInput").ap()
    d_xqT = nc.dram_tensor("xqT", [DM, SQ], bf16, kind="ExternalInput").ap()
    d_wq = nc.dram_tensor("wq", [DM, 4, 128], bf16, kind="ExternalInput").ap()
    d_wk = nc.dram_tensor("wk", [DM, 4, 128], bf16, kind="ExternalInput").ap()
    d_wv = nc.dram_tensor("wv", [DM, DK], bf16, kind="ExternalInput").ap()
    d_bq = nc.dram_tensor("bq", [128, 4], f32, kind="ExternalInput").ap()
    d_bk = nc.dram_tensor("bk", [128, 4], f32, kind="ExternalInput").ap()
    d_sel = nc.dram_tensor("sel", [128, NH], f32, kind="ExternalInput").ap()
    d_rep = nc.dram_tensor("rep", [NH, 128], f32, kind="ExternalInput").ap()
    d_wo = nc.dram_tensor("wo", [128, DM], bf16, kind="ExternalInput").ap()
    d_be = nc.dram_tensor("be", [1, DM], f32, kind="ExternalInput").ap()
    d_y = nc.dram_tensor("y", [SQ, DM], f32, kind="ExternalOutput").ap()

    with tile.TileContext(nc) as tc:
        with tc.tile_pool(name="const", bufs=1) as cp:
            # ---- load inputs to SBUF ----
            xT_sb = cp.tile([128, 8, S], bf16)
            xqT_sb = cp.tile([128, 8, SQ], bf16)
            wq_sb = cp.tile([128, 8, 4, 128], bf16)
            wk_sb = cp.tile([128, 8, 4, 128], bf16)
            wv_sb = cp.tile([128, 8, DK], bf16)
            for kc in range(8):
                r = slice(kc * 128, (kc + 1) * 128)
                nc.sync.dma_start(out=xT_sb[:, kc, :], in_=d_xT[r, :])
                nc.sync.dma_start(out=xqT_sb[:, kc, :], in_=d_xqT[r, :])
                nc.sync.dma_start(out=wq_sb[:, kc, :, :], in_=d_wq[r, :, :])
                nc.sync.dma_start(out=wk_sb[:, kc, :, :], in_=d_wk[r, :, :])
                nc.sync.dma_start(out=wv_sb[:, kc, :], in_=d_wv[r, :])
            bq_sb = cp.tile([128, 4], f32)
            bk_sb = cp.tile([128, 4], f32)
            sel_sb = cp.tile([128, NH], f32)
            rep_sb = cp.tile([NH, 128], f32)
            wo_sb = cp.tile([128, DM], bf16)
            be_sb = cp.tile([128, DM], f32)
            nc.sync.dma_start(out=bq_sb, in_=d_bq)
            nc.sync.dma_start(out=bk_sb, in_=d_bk)
            nc.sync.dma_start(out=sel_sb, in_=d_sel)
            nc.sync.dma_start(out=rep_sb, in_=d_rep)
            nc.sync.dma_start(out=wo_sb, in_=d_wo)
            be_b = bass.AP(tensor=d_be.tensor, offset=d_be.offset,
                           ap=[[0, 128], [1, DM]])
            nc.sync.dma_start(out=be_sb, in_=be_b)

            qT = cp.tile([128, 4, SQ], bf16)     # strip g: heads 4g..4g+3
            kT = cp.tile([128, 4, S], bf16)
            v_aug = cp.tile([128, 16, 512], bf16)  # per l-chunk, per head 32w
            outT_sb = cp.tile([128, SQ], f32)

            nc.vector.memset(v_aug, 0.0)
            va4 = v_aug.rearrange("p c (j q) -> p c j q", j=4)
            for s in range(4):
                nc.vector.memset(va4[:, :, :, 40 * s + 4:40 * s + 5], 1.0)

            # ---- projections ----
            with tc.tile_pool(name="pp", bufs=2, space="PSUM") as pp, \
                 tc.tile_pool(name="pv", bufs=2, space="PSUM") as pvp:
                for g in range(4):
                    pt = pp.tile([128, SQ], f32)
                    for nh in range(2):
                        for kc in range(8):
                            nc.tensor.matmul(
                                pt[:, nh * 512:(nh + 1) * 512],
                                wq_sb[:, kc, g, :],
                                xqT_sb[:, kc, nh * 512:(nh + 1) * 512],
                                start=(kc == 0), stop=(kc == 7))
                    nc.vector.tensor_scalar_add(qT[:, g, :], pt, bq_sb[:, g:g + 1])
                for g in range(4):
                    for sh in range(2):
                        pt = pp.tile([128, 1024], f32)
                        for nh in range(2):
                            for kc in range(8):
                                nc.tensor.matmul(
                                    pt[:, nh * 512:(nh + 1) * 512],
                                    wk_sb[:, kc, g, :],
                                    xT_sb[:, kc, sh * 1024 + nh * 512: sh * 1024 + (nh + 1) * 512],
                                    start=(kc == 0), stop=(kc == 7))
                        nc.vector.tensor_scalar_add(
                            kT[:, g, sh * 1024:(sh + 1) * 1024], pt, bk_sb[:, g:g + 1])
                for c in range(16):
                    pv = pvp.tile([128, DK], f32)
                    for kc in range(8):
                        nc.tensor.matmul(
                            pv, xT_sb[:, kc, c * 128:(c + 1) * 128],
                            wv_sb[:, kc, :], start=(kc == 0), stop=(kc == 7))
                    pvr = pv.rearrange("p (j r) -> p j r", j=4)
                    for s in range(4):
                        nc.vector.tensor_copy(
                            va4[:, c, :, 40 * s:40 * s + 4],
                            pvr[:, :, 4 * s:4 * s + 4])

            # ---- attention main loop ----
            with tc.tile_pool(name="op", bufs=1, space="PSUM") as op, \
                 tc.tile_pool(name="sp", bufs=3, space="PSUM") as sp, \
                 tc.tile_pool(name="ep", bufs=6) as ep:
                oT = op.tile([128, SQ], f32)
                for j in range(4):
                    for c in range(16):
                        sts, ets = [], []
                        for s in range(4):
                            st = sp.tile([128, 1024], f32, tag="st")
                            et = ep.tile([128, 1024], bf16, tag="et")
                            sts.append(st)
                            ets.append(et)
                            for nh in range(2):
                                nc.tensor.matmul(
                                    st[:, nh * 512:(nh + 1) * 512],
                                    kT[32 * s:32 * s + 4, j, c * 128:(c + 1) * 128],
                                    qT[32 * s:32 * s + 4, j, nh * 512:(nh + 1) * 512],
                                    start=True, stop=True,
                                    tile_position=(32 * s, 0))
                        for s in range(4):
                            nc.scalar.activation(ets[s], sts[s], Exp)
                        for s in range(4):
                            h = 4 * j + s
                            for nh in range(2):
                                nc.tensor.matmul(
                                    oT[32 * j:32 * j + 32, nh * 512:(nh + 1) * 512],
                                    v_aug[:, c, 32 * h:32 * h + 32],
                                    ets[s][:, nh * 512:(nh + 1) * 512],
                                    start=(c == 0 and s == 0),
                                    stop=(c == 15 and s == 3),
                                    tile_position=(0, 32 * j))
                    nc.vector.tensor_copy(outT_sb[32 * j:32 * j + 32, :],
                                          oT[32 * j:32 * j + 32, :])

            # ---- normalize + output projection ----
            with tc.tile_pool(name="fp", bufs=2, space="PSUM") as fp, \
                 tc.tile_pool(name="np_", bufs=1, space="PSUM") as npp, \
                 tc.tile_pool(name="fs", bufs=2) as fs:
                dn = npp.tile([NH, SQ], f32)
                for nh in range(2):
                    nc.tensor.matmul(dn[:, nh * 512:(nh + 1) * 512], sel_sb,
                                     outT_sb[:, nh * 512:(nh + 1) * 512],
                                     start=True, stop=True)
                rc = cp.tile([NH, SQ], f32)
                nc.vector.reciprocal(rc, dn)
                rp = npp.tile([128, SQ], f32)
                for nh in range(2):
                    nc.tensor.matmul(rp[:, nh * 512:(nh + 1) * 512], rep_sb,
                                     rc[:, nh * 512:(nh + 1) * 512],
                                     start=True, stop=True)
                nrm = cp.tile([128, SQ], bf16)
                nc.vector.tensor_mul(nrm, outT_sb, rp)
                for m in range(8):
                    pf = fp.tile([128, DM], f32)
                    for nd in range(2):
                        nc.tensor.matmul(pf[:, nd * 512:(nd + 1) * 512],
                                         nrm[:, m * 128:(m + 1) * 128],
                                         wo_sb[:, nd * 512:(nd + 1) * 512],
                                         start=True, stop=True)
                    fo = fs.tile([128, DM], f32)
                    nc.vector.tensor_add(fo, pf, be_sb)
                    nc.sync.dma_start(out=d_y[m * 128:(m + 1) * 128, :], in_=fo)

    nc.compile()
    return nc


def _prep_consts(w_in, b_in, w_out, b_out):
    wq = w_in[0:64].astype(np.float64) / 8.0
    wk = w_in[64:128].astype(np.float64)
    wv = w_in[128:192]
    bq = b_in[0:64].astype(np.float64) / 8.0
    bk = b_in[64:128]
    bv = b_in[128:192]

    # strip-layout padded projection weights: head h=4g+s dim d ->
    # column 32s+d of group g
    wq_p = np.zeros((DM, 4, 128), F32)
    wk_p = np.zeros((DM, 4, 128), F32)
    bq_p = np.zeros((128, 4), F32)
    bk_p = np.zeros((128, 4), F32)
    for g in range(4):
        for s in range(4):
            h = 4 * g + s
            for d in range(HD):
                wq_p[:, g, 32 * s + d] = wq[4 * h + d]
                wk_p[:, g, 32 * s + d] = wk[4 * h + d]
                bq_p[32 * s + d, g] = bq[4 * h + d]
                bk_p[32 * s + d, g] = bk[4 * h + d]

    sel = np.zeros((128, NH), F32)
    rep = np.zeros((NH, 128), F32)
    wo = np.zeros((128, DM), F32)
    for h in range(NH):
        base = _slot(h)
        sel[base + 4, h] = 1.0
        for q in range(5):
            rep[h, base + q] = 1.0
        for d in range(HD):
            wo[base + d, :] = w_out[:, 4 * h + d]
    be = (b_out.astype(np.float64) + w_out.astype(np.float64) @ bv.astype(np.float64))

    return {
        "wq": wq_p.astype(BF16), "wk": wk_p.astype(BF16),
        "wv": wv.T.astype(BF16),
        "bq": bq_p.astype(F32), "bk": bk_p.astype(F32),
        "sel": sel, "rep": rep, "wo": wo.astype(BF16),
        "be": be.astype(F32).reshape(1, DM),
    }


def kernel(x, w_in, b_in, w_out, b_out, _trace=False, **kw):
    x = np.asarray(x, F32)
    consts = _prep_consts(np.asarray(w_in, F32), np.asarray(b_in, F32),
                          np.asarray(w_out, F32), np.asarray(b_out, F32))
    if "nc" not in _cache:
        _cache["nc"] = _build_nc()
    nc = _cache["nc"]

    xTs = [np.ascontiguousarray(x[b].T).astype(BF16) for b in range(B)]
    in_maps = []
    for core in range(NC_CORES):
        b, half = divmod(core, 2)
        m = dict(consts)
        m["xT"] = xTs[b]
        m["xqT"] = np.ascontiguousarray(xTs[b][:, half * SQ:(half + 1) * SQ])
        in_maps.append(m)

    res = run_bass_kernel_spmd(nc, in_maps, list(range(NC_CORES)),
                               trace=_trace)
    out = np.empty((B, S, DM), F32)
    for core in range(NC_CORES):
        b, half = divmod(core, 2)
        out[b, half * SQ:(half + 1) * SQ, :] = res.results[core]["y"]
    if _trace:
        return out, res
    return out

